# revision 6
# baseline (speedup 1.0000x reference)
"""Sharded cosine-similarity top-k retrieval kernel for 8 Trainium2 cores.

Problem: query [32], keys [1M, 32], values [1M, 512], k=64.
  sims = (keys @ q) / (max(||k||,eps) * max(||q||,eps))
  top64 = top_k(sims, 64); mask = top_sims >= 0.1
  out_values = values[idx] * mask[:, None]; out_sims = where(mask, top_sims, 0)

Sharding: keys/values row-wise, 125000 rows per core.  Each core computes
local sims + local top-64, AllGather(8x64 candidate sims) -> final top-64
(computed redundantly on every core), max_index search recovers local row
indices, AllGather(indices + count certificate) -> global rows; each core
gathers the value rows it owns via indirect DMA and masks; host sums the
disjoint partials.

Exactness certificate computed on device: count(sims >= 64th value) == 64
globally AND no duplicate values among the final 64.  If it fails (fp32
value collision in the top set, probability ~1e-6) the host falls back to
an exact numpy implementation.
"""

import os
import numpy as np

P = 128          # SBUF partitions
D = 32           # key dim
NCORES = 8
EPS = 1e-8
THRESHOLD = 0.1
NEG = -1.0e30

# full-size problem constants
N_FULL = 1_000_000
NLOC = N_FULL // NCORES          # 125000 rows per core
COLS = 1024                      # sims layout [128, COLS]; local row = p*COLS + j
NPAD = P * COLS                  # 131072 padded key rows per core
DV = 512                         # value dim
K = 64


def build_nc(nloc=NLOC, cols=COLS, dval=DV, ncores=NCORES, chunk_cols=128):
    """Build the Bass module (same NEFF for all cores)."""
    import concourse.bacc as bacc
    import concourse.bass as bass
    import concourse.mybir as mybir
    import concourse.tile as tile
    from concourse.masks import make_identity

    npad = P * cols
    f32 = mybir.dt.float32
    i32 = mybir.dt.int32
    u32 = mybir.dt.uint32

    nc = bacc.Bacc(
        "TRN2",
        target_bir_lowering=False,
        debug=False,
        num_devices=ncores,
    )

    query_t = nc.dram_tensor("query", [1, D], f32, kind="ExternalInput")
    keys_t = nc.dram_tensor("keys", [npad, D], f32, kind="ExternalInput")
    values_t = nc.dram_tensor("values", [nloc, dval], f32, kind="ExternalInput")
    out_values_t = nc.dram_tensor("out_values", [K, dval], f32, kind="ExternalOutput")
    out_sims_t = nc.dram_tensor("out_sims", [1, K], f32, kind="ExternalOutput")
    out_flag_t = nc.dram_tensor("out_flag", [1, 1], f32, kind="ExternalOutput")

    AX = mybir.AxisListType
    OP = mybir.AluOpType
    ACT = mybir.ActivationFunctionType

    with tile.TileContext(nc) as tc:
        with (
            tc.tile_pool(name="persist", bufs=1) as pp,
            tc.tile_pool(name="ktiles", bufs=2) as kp,
            tc.tile_pool(name="tmp", bufs=2) as tp,
            tc.tile_pool(name="small", bufs=1) as sp,
            tc.tile_pool(name="psum", bufs=2, space="PSUM") as psp,
            tc.tile_pool(name="dram", bufs=1, space="DRAM") as dp,
        ):
            # ---- A. query prep -------------------------------------------
            qsb = sp.tile([1, D], f32)
            nc.sync.dma_start(out=qsb[:], in_=query_t[0:1, :])
            qsq = sp.tile([1, D], f32)
            nc.vector.tensor_mul(qsq[:], qsb[:], qsb[:])
            q2 = sp.tile([1, 1], f32)
            nc.vector.tensor_reduce(out=q2[:], in_=qsq[:], axis=AX.X, op=OP.add)
            qn = sp.tile([1, 1], f32)
            nc.scalar.activation(qn[:], q2[:], ACT.Sqrt)
            nc.vector.tensor_scalar_max(qn[:], qn[:], float(EPS))
            iqn = sp.tile([1, 1], f32)
            nc.vector.reciprocal(iqn[:], qn[:])
            qp = sp.tile([1, D], f32)
            nc.vector.tensor_scalar_mul(qp[:], qsb[:], iqn[:, 0:1])
            # broadcast q' to all partitions, shaped [P, 1, D] for free-broadcast
            qall = pp.tile([P, 1, D], f32)
            nc.gpsimd.partition_broadcast(qall[:, 0, :], qp[:])

            # ---- B. scan keys, accumulate dots and sum-of-squares --------
            dots = pp.tile([P, cols], f32)
            k2 = pp.tile([P, cols], f32)
            keys_view = keys_t.ap().rearrange("(p j) d -> p j d", p=P)
            n_chunks = (cols + chunk_cols - 1) // chunk_cols
            for ci in range(n_chunks):
                j0 = ci * chunk_cols
                jc = min(chunk_cols, cols - j0)
                kt = kp.tile([P, chunk_cols, D], f32, tag="kt")
                nc.sync.dma_start(
                    out=kt[:, :jc, :], in_=keys_view[:, j0 : j0 + jc, :]
                )
                prod = tp.tile([P, chunk_cols, D], f32, tag="prod")
                nc.vector.tensor_mul(
                    prod[:, :jc, :],
                    kt[:, :jc, :],
                    qall[:].to_broadcast([P, jc, D]),
                )
                nc.vector.tensor_reduce(
                    out=dots[:, j0 : j0 + jc],
                    in_=prod[:, :jc, :],
                    axis=AX.X,
                    op=OP.add,
                )
                sq = tp.tile([P, chunk_cols, D], f32, tag="sq")
                nc.scalar.activation(sq[:, :jc, :], kt[:, :jc, :], ACT.Square)
                nc.vector.tensor_reduce(
                    out=k2[:, j0 : j0 + jc],
                    in_=sq[:, :jc, :],
                    axis=AX.X,
                    op=OP.add,
                )

            # ---- C. sims = dots * (1/max(sqrt(k2),eps)) ------------------
            kn = pp.tile([P, cols], f32)
            nc.scalar.activation(kn[:], k2[:], ACT.Sqrt)
            nc.vector.tensor_scalar_max(kn[:], kn[:], float(EPS))
            ikn = pp.tile([P, cols], f32)
            nc.vector.reciprocal(ikn[:], kn[:])
            sims = pp.tile([P, cols], f32)
            nc.vector.tensor_mul(sims[:], dots[:], ikn[:])
            # padded key rows are -query on the host side: cosine == -1,
            # so they can never enter the top-64 and need no masking here.

            ident = pp.tile([P, P], f32)
            make_identity(nc, ident[:])

            # ---- D. local top-64 values ----------------------------------
            t8 = sp.tile([P, 8], f32)
            nc.vector.max(out=t8[:], in_=sims[:])
            t8_ps = psp.tile([8, P], f32, space="PSUM")
            nc.tensor.transpose(out=t8_ps[:], in_=t8[:], identity=ident[:])
            c8 = sp.tile([8, P], f32)
            nc.vector.tensor_copy(c8[:], t8_ps[:])
            # stage 1: per-row top-64 of [8, 128]
            c64 = sp.tile([8, K], f32)
            work8 = sp.tile([8, P], f32)
            src = c8
            for r in range(8):
                nc.vector.max(out=c64[:, 8 * r : 8 * r + 8], in_=src[:])
                nc.vector.match_replace(
                    out=work8[:],
                    in_to_replace=c64[:, 8 * r : 8 * r + 8],
                    in_values=src[:],
                    imm_value=NEG,
                )
                src = work8
            # stage 2: reshape [8,64] -> [1,512] via DRAM, then top-64
            dr_c64 = dp.tile([8, K], f32)
            nc.sync.dma_start(out=dr_c64[:], in_=c64[:])
            cm = sp.tile([1, 8 * K], f32)
            nc.sync.dma_start(
                out=cm[:], in_=dr_c64[:].rearrange("a b -> (a b)")[None, :]
            )
            l64 = sp.tile([1, K], f32)
            workl = sp.tile([1, 8 * K], f32)
            src = cm
            for r in range(8):
                nc.vector.max(out=l64[:, 8 * r : 8 * r + 8], in_=src[:])
                nc.vector.match_replace(
                    out=workl[:],
                    in_to_replace=l64[:, 8 * r : 8 * r + 8],
                    in_values=src[:],
                    imm_value=NEG,
                )
                src = workl

            # ---- F1. AllGather candidate sims ----------------------------
            rg = [list(range(ncores))]
            ag1_in = dp.tile([1, K], f32)
            nc.sync.dma_start(out=ag1_in[:], in_=l64[:])
            ag1_out = dp.tile([ncores, K], f32)
            nc.gpsimd.collective_compute(
                "AllGather",
                OP.bypass,
                replica_groups=rg,
                ins=[ag1_in.opt()],
                outs=[ag1_out.opt()],
            )
            g512 = sp.tile([1, ncores * K], f32)
            nc.sync.dma_start(
                out=g512[:], in_=ag1_out[:].rearrange("a b -> (a b)")[None, :]
            )

            # ---- F2. final top-64 (redundant on every core) --------------
            f64 = sp.tile([1, K], f32)
            workf = sp.tile([1, ncores * K], f32)
            src = g512
            for r in range(8):
                nc.vector.max(out=f64[:, 8 * r : 8 * r + 8], in_=src[:])
                nc.vector.match_replace(
                    out=workf[:],
                    in_to_replace=f64[:, 8 * r : 8 * r + 8],
                    in_values=src[:],
                    imm_value=NEG,
                )
                src = workf

            # ---- E. local index search -----------------------------------
            f64b = sp.tile([P, K], f32)
            nc.gpsimd.partition_broadcast(f64b[:], f64[:])
            idxs = sp.tile([P, K], u32)
            for g in range(K // 8):
                nc.vector.max_index(
                    out=idxs[:, 8 * g : 8 * g + 8],
                    in_max=f64b[:, 8 * g : 8 * g + 8],
                    in_values=sims[:],
                )
            idxf = sp.tile([P, K], f32)
            nc.vector.tensor_copy(idxf[:], idxs[:])
            pbase_i = sp.tile([P, 1], i32)
            nc.gpsimd.iota(pbase_i[:], [[0, 1]], channel_multiplier=cols)
            pbase = sp.tile([P, 1], f32)
            nc.vector.tensor_copy(pbase[:], pbase_i[:])
            lrows = sp.tile([P, K], f32)
            nc.vector.tensor_add(
                lrows[:], idxf[:], pbase[:].to_broadcast([P, K])
            )
            lr_ps = psp.tile([K, P], f32, space="PSUM")
            nc.tensor.transpose(out=lr_ps[:], in_=lrows[:], identity=ident[:])
            lmin = sp.tile([K, 1], f32)
            nc.vector.tensor_reduce(
                out=lmin[:], in_=lr_ps[:], axis=AX.X, op=OP.min
            )
            # global row = local row + core_id * nloc
            pid_sb = sp.tile([1, 1], u32)
            nc.sync.dma_start(
                out=pid_sb[:], in_=nc.partition_id_tensor[0:1, 0:1]
            )
            pidf = sp.tile([1, 1], f32)
            nc.vector.tensor_copy(pidf[:], pid_sb[:])
            base1 = sp.tile([1, 1], f32)
            nc.vector.tensor_scalar_mul(base1[:], pidf[:], float(nloc))
            base64 = sp.tile([K, 1], f32)
            nc.gpsimd.partition_broadcast(base64[:], base1[:])
            grow = sp.tile([K, 1], f32)
            nc.vector.tensor_add(grow[:], lmin[:], base64[:])

            # ---- F8. certificate: count(sims >= min(f64)) and dups ------
            tf_b = sp.tile([P, 1], f32)
            nc.gpsimd.partition_broadcast(tf_b[:], f64[:, K - 1 : K])
            ge = pp.tile([P, cols], f32)
            nc.vector.tensor_scalar(
                ge[:], sims[:], tf_b[:, 0:1], None, op0=OP.is_ge
            )
            cnt = sp.tile([P, 1], f32)
            nc.vector.tensor_reduce(out=cnt[:], in_=ge[:], axis=AX.X, op=OP.add)
            ones128 = sp.tile([P, 1], f32)
            nc.vector.memset(ones128[:], 1.0)
            cnt_ps = psp.tile([1, 1], f32, space="PSUM")
            nc.tensor.matmul(
                out=cnt_ps[:], lhsT=cnt[:], rhs=ones128[:], start=True, stop=True
            )
            cnt_sb = sp.tile([1, 1], f32)
            nc.vector.tensor_copy(cnt_sb[:], cnt_ps[:])

            # ---- F3. AllGather rows + counts -----------------------------
            ag2_in = dp.tile([K + 1, 1], f32)
            nc.sync.dma_start(out=ag2_in[0:K, :], in_=grow[:])
            nc.sync.dma_start(out=ag2_in[K : K + 1, :], in_=cnt_sb[:])
            ag2_out = dp.tile([ncores * (K + 1), 1], f32)
            nc.gpsimd.collective_compute(
                "AllGather",
                OP.bypass,
                replica_groups=rg,
                ins=[ag2_in.opt()],
                outs=[ag2_out.opt()],
            )
            # rows part -> [K, ncores]; counts part -> [1, ncores]
            g_rows = sp.tile([K, ncores], f32)
            ag2_view = ag2_out[:, 0].rearrange("(c s) -> s c", s=K + 1)
            nc.sync.dma_start(out=g_rows[:], in_=ag2_view[0:K, :])
            g_cnts = sp.tile([1, ncores], f32)
            nc.sync.dma_start(out=g_cnts[:], in_=ag2_view[K : K + 1, :])
            gmin = sp.tile([K, 1], f32)
            nc.vector.tensor_reduce(
                out=gmin[:], in_=g_rows[:], axis=AX.X, op=OP.min
            )
            cnt_tot = sp.tile([1, 1], f32)
            nc.vector.tensor_reduce(
                out=cnt_tot[:], in_=g_cnts[:], axis=AX.X, op=OP.add
            )
            flag_cnt = sp.tile([1, 1], f32)
            nc.vector.tensor_scalar(
                flag_cnt[:], cnt_tot[:], float(K), None, op0=OP.is_equal
            )
            # duplicate values among final 64?
            dup = sp.tile([1, K - 1], f32)
            nc.vector.tensor_tensor(
                out=dup[:], in0=f64[:, 0 : K - 1], in1=f64[:, 1:K], op=OP.is_equal
            )
            dupc = sp.tile([1, 1], f32)
            nc.vector.tensor_reduce(out=dupc[:], in_=dup[:], axis=AX.X, op=OP.add)
            flag_dup = sp.tile([1, 1], f32)
            nc.vector.tensor_scalar(
                flag_dup[:], dupc[:], 0.5, None, op0=OP.is_lt
            )
            flag = sp.tile([1, 1], f32)
            nc.vector.tensor_mul(flag[:], flag_cnt[:], flag_dup[:])
            nc.sync.dma_start(out=out_flag_t[0:1, :], in_=flag[:])

            # ---- F4-F7. ownership, gather, masks, outputs ----------------
            lidx = sp.tile([K, 1], f32)
            nc.vector.tensor_sub(lidx[:], gmin[:], base64[:])
            own_lo = sp.tile([K, 1], f32)
            nc.vector.tensor_scalar(own_lo[:], lidx[:], -0.5, None, op0=OP.is_gt)
            own_hi = sp.tile([K, 1], f32)
            nc.vector.tensor_scalar(
                own_hi[:], lidx[:], float(nloc) - 0.5, None, op0=OP.is_lt
            )
            own = sp.tile([K, 1], f32)
            nc.vector.tensor_mul(own[:], own_lo[:], own_hi[:])
            # sims per slot on K partitions: transpose f64 via matmul with ones
            ones11 = sp.tile([1, 1], f32)
            nc.vector.memset(ones11[:], 1.0)
            s64_ps = psp.tile([K, 1], f32, space="PSUM")
            nc.tensor.matmul(
                out=s64_ps[:], lhsT=f64[:], rhs=ones11[:], start=True, stop=True
            )
            th64 = sp.tile([K, 1], f32)
            nc.vector.tensor_scalar(
                th64[:], s64_ps[:], float(THRESHOLD), None, op0=OP.is_ge
            )
            msk = sp.tile([K, 1], f32)
            nc.vector.tensor_mul(msk[:], own[:], th64[:])
            # clamp local idx and gather
            lidx_cl = sp.tile([K, 1], f32)
            nc.vector.tensor_scalar_max(lidx_cl[:], lidx[:], 0.0)
            nc.vector.tensor_scalar_min(lidx_cl[:], lidx_cl[:], float(nloc - 1))
            idx_i = sp.tile([K, 1], i32)
            nc.vector.tensor_copy(idx_i[:], lidx_cl[:])
            import concourse.bass as bass_mod

            vals = sp.tile([K, dval], f32)
            nc.gpsimd.indirect_dma_start(
                out=vals[:],
                out_offset=None,
                in_=values_t[:, :],
                in_offset=bass_mod.IndirectOffsetOnAxis(ap=idx_i[:, 0:1], axis=0),
            )
            outv = sp.tile([K, dval], f32)
            nc.vector.tensor_scalar_mul(outv[:], vals[:], msk[:, 0:1])
            nc.sync.dma_start(out=out_values_t[:, :], in_=outv[:])
            # out_sims = f64 * (f64 >= thr)
            thr_row = sp.tile([1, K], f32)
            nc.vector.tensor_scalar(
                thr_row[:], f64[:], float(THRESHOLD), None, op0=OP.is_ge
            )
            osims = sp.tile([1, K], f32)
            nc.vector.tensor_mul(osims[:], f64[:], thr_row[:])
            nc.sync.dma_start(out=out_sims_t[0:1, :], in_=osims[:])

    nc.compile()
    return nc


# ---------------------------------------------------------------------------
# host side
# ---------------------------------------------------------------------------

_NC_CACHE = {}
LAST_RESULT = None


def _get_nc():
    key = "full"
    if key not in _NC_CACHE:
        _NC_CACHE[key] = build_nc()
    return _NC_CACHE[key]


def _shard_inputs(query, keys, values):
    q2d = np.ascontiguousarray(query.reshape(1, D).astype(np.float32))
    in_maps = []
    for c in range(NCORES):
        ks = np.empty((NPAD, D), dtype=np.float32)
        ks[:NLOC] = keys[c * NLOC : (c + 1) * NLOC]
        ks[NLOC:] = -query.reshape(1, D)
        vs = np.ascontiguousarray(values[c * NLOC : (c + 1) * NLOC])
        in_maps.append({"query": q2d, "keys": ks, "values": vs})
    return in_maps


def _numpy_reference(query, keys, values, k):
    """Exact fallback mirroring reference.py in numpy float32."""
    q = query.astype(np.float32)
    ks = keys.astype(np.float32)
    q_norm = max(np.sqrt(np.sum(q * q, dtype=np.float32)), EPS)
    k_norm = np.maximum(np.sqrt(np.sum(ks * ks, axis=-1, dtype=np.float32)), EPS)
    sims = (ks @ q) / (k_norm * np.float32(q_norm))
    idx = np.argsort(-sims, kind="stable")[:k]
    top = sims[idx]
    mask = top >= THRESHOLD
    out_values = values[idx].astype(np.float32) * mask[:, None].astype(np.float32)
    out_sims = np.where(mask, top, np.zeros_like(top)).astype(np.float32)
    return out_values, out_sims


def kernel(query, keys, values, k):
    query = np.asarray(query, dtype=np.float32)
    keys = np.asarray(keys, dtype=np.float32)
    values = np.asarray(values, dtype=np.float32)
    k = int(k)
    if (
        k != K
        or keys.shape != (N_FULL, D)
        or values.shape != (N_FULL, DV)
        or query.shape != (D,)
    ):
        return _numpy_reference(query, keys, values, k)

    from concourse.bass_utils import run_bass_kernel_spmd

    nc = _get_nc()
    in_maps = _shard_inputs(query, keys, values)
    res = run_bass_kernel_spmd(nc, in_maps, core_ids=list(range(NCORES)))
    global LAST_RESULT
    LAST_RESULT = res
    results = res.results
    flags = [float(r["out_flag"][0, 0]) for r in results]
    if not all(f > 0.5 for f in flags):
        return _numpy_reference(query, keys, values, k)
    out_values = np.zeros((K, DV), dtype=np.float32)
    for r in results:
        out_values += r["out_values"]
    out_sims = results[0]["out_sims"].reshape(K).copy()
    return out_values, out_sims
